# revision 6
# baseline (speedup 1.0000x reference)
"""Trainium2 Bass kernel: per-pixel 5x5-patch channel covariance.

R[b,h,w,k,l] = (1/N) sum_n (p_kn - mu_k)(p_ln - mu_l)   (N=25, reflect pad)

Identity used:  R = box5x5(S_k * S_l)/25 - mu_k * mu_l,  mu = box5x5(S)/25.
The separable box sums run as banded matmuls on TensorE; reflect padding is
folded into the band-matrix weights (entries in {0,1,2}), so no spatial
padding is materialized.  Host pre-scales S by 1/5 so the two band passes
(weights 1.0, exact in bf16) produce box/25 directly.

Sharding: 8 cores = 4 batches x 2 H-halves.  Fully data parallel.
"""
import sys

sys.path.insert(0, "/opt/trn_rl_repo")

from contextlib import ExitStack

import numpy as np

import concourse.bacc as bacc
import concourse.mybir as mybir
import concourse.tile as tile
from concourse import bass_utils

B, K, H, W = 4, 16, 256, 256
HH = 128           # output rows per core
SR = 132           # shard rows (128 + 2 halo each side, edge-clamped)
PAIRS = [(k, l) for k in range(K) for l in range(k, K)]
NPAIR = len(PAIRS)         # 136
NCH = K + NPAIR            # 152 channels: 0..15 mean, 16.. pairs
NOCT = NCH // 8            # 19 channel octets
F32 = mybir.dt.float32
BF16 = mybir.dt.bfloat16


def _reflect_idx(i, n):
    if i < 0:
        return -i
    if i >= n:
        return 2 * (n - 1) - i
    return i


def _build_bw():
    """[256 w'col, 256 wout] box weights with reflection folded; -> [128, 4*128]
    blocks indexed (oh, chunk): BW[:, (oh*2+c)*128 + wl] = M[c*128 + :, oh*128 + wl]."""
    M = np.zeros((W, W), dtype=np.float32)
    for w in range(W):
        for j in range(5):
            M[_reflect_idx(w - 2 + j, W), w] += 1.0
    out = np.zeros((128, 512), dtype=np.float32)
    for oh in range(2):
        for c in range(2):
            out[:, (oh * 2 + c) * 128:(oh * 2 + c) * 128 + 128] = \
                M[c * 128:(c + 1) * 128, oh * 128:(oh + 1) * 128]
    return out


def _build_br(half):
    """[68, 128]: cols rt*64+hl; rows are shard-local rows within row-tile rt."""
    hbase = half * HH
    M = np.zeros((68, 128), dtype=np.float32)
    for rt in range(2):
        for hl in range(64):
            hg = hbase + rt * 64 + hl
            for i in range(5):
                r = _reflect_idx(hg - 2 + i, H)
                j = r + 2 - hbase          # canonical shard row
                M[j - rt * 64, rt * 64 + hl] += 1.0
    return M


def _ksegs_in_octet(oct_idx):
    """Pair channels live at ch 16..151. For a channel octet [oct*8, oct*8+8),
    return list of (j0, k, l0, nl): local offset j0, channel k, first l, count."""
    lo, hi = oct_idx * 8, oct_idx * 8 + 8
    segs = []
    p = 0
    for k in range(K):
        n = K - k
        # pairs (k, k..15) occupy ch [16+p, 16+p+n)
        s, e = 16 + p, 16 + p + n
        a, b = max(lo, s), min(hi, e)
        if a < b:
            segs.append((a - lo, k, k + (a - s), b - a))
        p += n
    return segs


def _build_kernel():
    nc = bacc.Bacc("TRN2", target_bir_lowering=False, debug=False)
    S_d = nc.dram_tensor("S", [SR, K, W], BF16, kind="ExternalInput").ap()
    BR_d = nc.dram_tensor("BR", [68, 128], BF16, kind="ExternalInput").ap()
    BW_d = nc.dram_tensor("BW", [128, 512], BF16, kind="ExternalInput").ap()
    R_d = nc.dram_tensor("R", [HH, W, K * K], BF16, kind="ExternalOutput").ap()

    with tile.TileContext(nc) as tc, ExitStack() as ctx:
        const_p = ctx.enter_context(tc.tile_pool(name="const", bufs=1))
        sp_p = ctx.enter_context(tc.tile_pool(name="sp", bufs=1))
        t_p = ctx.enter_context(tc.tile_pool(name="tprod", bufs=2))
        i1_p = ctx.enter_context(tc.tile_pool(name="i1", bufs=2))
        mu_p = ctx.enter_context(tc.tile_pool(name="mu", bufs=2))
        m_p = ctx.enter_context(tc.tile_pool(name="mm", bufs=1))
        r_p = ctx.enter_context(tc.tile_pool(name="rout", bufs=2))
        ps1_p = ctx.enter_context(tc.tile_pool(name="ps1", bufs=2, space="PSUM"))
        ps2_p = ctx.enter_context(tc.tile_pool(name="ps2", bufs=4, space="PSUM"))

        br = const_p.tile([68, 128], BF16)
        bw = const_p.tile([128, 512], BF16)
        nc.sync.dma_start(br[:], BR_d)
        nc.sync.dma_start(bw[:], BW_d)

        sp0 = sp_p.tile([68, K, W], BF16)
        sp1 = sp_p.tile([68, K, W], BF16)
        nc.sync.dma_start(sp0[:], S_d[0:68])
        nc.sync.dma_start(sp1[:], S_d[64:132])
        sps = [sp0, sp1]

        for rt in range(2):
            sp = sps[rt]
            brt = br[:, rt * 64:(rt + 1) * 64]
            i1c0 = i1_p.tile([128, NCH * 64], BF16, name="i1c0")
            i1c1 = i1_p.tile([128, NCH * 64], BF16, name="i1c1")
            for oc in range(NOCT):
                T = t_p.tile([68, 8, W], BF16, name="T")
                lo = oc * 8
                if lo < K:  # mean channels: plain copy
                    n = min(8, K - lo)
                    nc.vector.tensor_copy(T[:, 0:n, :], sp[:, lo:lo + n, :])
                for (j0, k, l0, nl) in _ksegs_in_octet(oc):
                    in0 = sp[:, k, :].unsqueeze(1).broadcast_to([68, nl, W])
                    nc.vector.tensor_mul(
                        T[:, j0:j0 + nl, :], in0, sp[:, l0:l0 + nl, :])
                psa = ps1_p.tile([128, 512], F32, name="psa")
                psb = ps1_p.tile([128, 512], F32, name="psb")
                Tf = T[:].rearrange("p a b -> p (a b)")
                for j in range(8):
                    nc.tensor.matmul(psa[:, j * 64:(j + 1) * 64],
                                     Tf[:, j * 256:j * 256 + 128], brt,
                                     start=True, stop=True)
                    nc.tensor.matmul(psb[:, j * 64:(j + 1) * 64],
                                     Tf[:, j * 256 + 128:j * 256 + 256], brt,
                                     start=True, stop=True)
                nc.scalar.copy(i1c0[:, oc * 512:(oc + 1) * 512], psa[:])
                nc.scalar.copy(i1c1[:, oc * 512:(oc + 1) * 512], psb[:])

            i0v = i1c0[:].rearrange("p (c h) -> p c h", c=NCH)
            i1v = i1c1[:].rearrange("p (c h) -> p c h", c=NCH)
            for oh in range(2):
                bwa = bw[:, (oh * 2) * 128:(oh * 2) * 128 + 128]
                bwb = bw[:, (oh * 2 + 1) * 128:(oh * 2 + 1) * 128 + 128]
                # --- mean channels -> mu (bf16, scaled by 1/5) ---
                mub = mu_p.tile([128, K, 64], BF16, name="mub")
                for mo in range(2):
                    ps2 = ps2_p.tile([128, 512], F32, name="ps2")
                    nc.tensor.matmul(ps2[:], bwa,
                                     i0v[:, mo * 8:(mo + 1) * 8, :],
                                     start=True, stop=False)
                    nc.tensor.matmul(ps2[:], bwb,
                                     i1v[:, mo * 8:(mo + 1) * 8, :],
                                     start=False, stop=True)
                    nc.scalar.mul(mub[:, mo * 8:(mo + 1) * 8, :],
                                  ps2[:].rearrange("p (c h) -> p c h", c=8), 0.2)
                # --- M = mu_k * mu_l ---
                M = m_p.tile([128, NPAIR, 64], BF16, name="M")
                p0 = 0
                for k in range(K):
                    nl = K - k
                    in0 = mub[:, k, :].unsqueeze(1).broadcast_to([128, nl, 64])
                    nc.vector.tensor_mul(M[:, p0:p0 + nl, :], in0,
                                         mub[:, k:K, :])
                    p0 += nl
                # --- pair channels: MM2, subtract, mirror ---
                rsb = r_p.tile([128, 64, K * K], BF16, name="rsb")
                for oc in range(2, NOCT):
                    ps2 = ps2_p.tile([128, 512], F32, name="ps2")
                    nc.tensor.matmul(ps2[:], bwa,
                                     i0v[:, oc * 8:(oc + 1) * 8, :],
                                     start=True, stop=False)
                    nc.tensor.matmul(ps2[:], bwb,
                                     i1v[:, oc * 8:(oc + 1) * 8, :],
                                     start=False, stop=True)
                    p2v = ps2[:].rearrange("p (c h) -> p c h", c=8)
                    for (j0, k, l0, nl) in _ksegs_in_octet(oc):
                        pr = (l0 - k) + (k * (2 * K + 1 - k)) // 2
                        # out positions kl = k*16 + l, l in [l0, l0+nl)
                        dst = rsb[:].rearrange("p h q -> p q h")[
                            :, k * K + l0:k * K + l0 + nl, :]
                        nc.vector.tensor_sub(dst, p2v[:, j0:j0 + nl, :],
                                             M[:, pr:pr + nl, :])
                # mirror lower triangle on gpsimd
                rq = rsb[:].rearrange("p h q -> p q h")
                rqv = rsb[:].rearrange("p h (a b) -> p a b h", a=K)
                for k in range(K - 1):
                    src = rq[:, k * K + k + 1:k * K + K, :]
                    dst = rqv[:, k + 1:K, k, :]
                    nc.gpsimd.tensor_copy(dst, src)
                # DMA out: R[rt*64 : +64, oh*128 : +128, :]
                dview = R_d[rt * 64:rt * 64 + 64,
                            oh * 128:(oh + 1) * 128, :].transpose([1, 0, 2])
                nc.sync.dma_start(dview, rsb[:])

    nc.compile()
    return nc


_NC_CACHE = {}


def _get_nc():
    if "nc" not in _NC_CACHE:
        _NC_CACHE["nc"] = _build_kernel()
    return _NC_CACHE["nc"]


def _prep_in_maps(S):
    S = np.asarray(S, dtype=np.float32)
    np_bf16 = mybir.dt.np(BF16)
    bw = _build_bw().astype(np_bf16)
    brs = [(_build_br(h)).astype(np_bf16) for h in range(2)]
    Ss = S * np.float32(0.2)
    in_maps = []
    for b in range(B):
        for half in range(2):
            hbase = half * HH
            rows = np.clip(np.arange(hbase - 2, hbase + 130), 0, H - 1)
            shard = Ss[b][:, rows, :].transpose(1, 0, 2)   # [132, K, 256]
            shard = np.ascontiguousarray(shard).astype(np_bf16)
            in_maps.append({"S": shard, "BR": brs[half], "BW": bw})
    return in_maps


def _assemble(results):
    out = np.empty((B, H, W, K, K), dtype=np.float32)
    for i in range(8):
        b, half = divmod(i, 2)
        r = np.asarray(results[i]["R"]).astype(np.float32)
        out[b, half * HH:(half + 1) * HH] = r.reshape(HH, W, K, K)
    return out


def kernel(S):
    """S: [4, 16, 256, 256] float32 -> R: [4, 256, 256, 16, 16] float32."""
    nc = _get_nc()
    in_maps = _prep_in_maps(S)
    res = bass_utils.run_bass_kernel_spmd(nc, in_maps, list(range(8)))
    return _assemble(res.results)


# revision 11
# speedup vs baseline: 104.1187x; 104.1187x over previous
"""Trainium2 Bass kernel: per-pixel 5x5-patch channel covariance.

R[b,h,w,k,l] = (1/N) sum_n (p_kn - mu_k)(p_ln - mu_l)   (N=25, reflect pad)

Identity used:  R = box5x5(S_k * S_l)/25 - mu_k * mu_l,  mu = box5x5(S)/25.
The separable box sums run as banded matmuls on TensorE; reflect padding is
folded into the band-matrix weights (entries in {0,1,2}), so no spatial
padding is materialized.  Host pre-scales S by 1/5 so the two band passes
(weights 1.0, exact in bf16) produce box/25 directly.

Sharding: 8 cores = 4 batches x 2 H-halves.  Fully data parallel.
"""
import sys

sys.path.insert(0, "/opt/trn_rl_repo")

from contextlib import ExitStack

import numpy as np

import concourse.bacc as bacc
import concourse.mybir as mybir
import concourse.tile as tile
from concourse import bass_utils

B, K, H, W = 4, 16, 256, 256
HH = 128           # output rows per core
SR = 132           # shard rows (128 + 2 halo each side, edge-clamped)
PAIRS = [(k, l) for k in range(K) for l in range(k, K)]
NPAIR = len(PAIRS)         # 136
NCH = K + NPAIR            # 152 channels: 0..15 mean, 16.. pairs
NOCT = NCH // 8            # 19 channel octets
F32 = mybir.dt.float32
BF16 = mybir.dt.bfloat16


def _reflect_idx(i, n):
    if i < 0:
        return -i
    if i >= n:
        return 2 * (n - 1) - i
    return i


def _build_bw():
    """[256 w'col, 256 wout] box weights with reflection folded; -> [128, 4*128]
    blocks indexed (oh, chunk): BW[:, (oh*2+c)*128 + wl] = M[c*128 + :, oh*128 + wl]."""
    M = np.zeros((W, W), dtype=np.float32)
    for w in range(W):
        for j in range(5):
            M[_reflect_idx(w - 2 + j, W), w] += 1.0
    out = np.zeros((128, 512), dtype=np.float32)
    for oh in range(2):
        for c in range(2):
            out[:, (oh * 2 + c) * 128:(oh * 2 + c) * 128 + 128] = \
                M[c * 128:(c + 1) * 128, oh * 128:(oh + 1) * 128]
    return out


def _build_br(half):
    """[68, 128]: cols rt*64+hl; rows are shard-local rows within row-tile rt."""
    hbase = half * HH
    M = np.zeros((68, 128), dtype=np.float32)
    for rt in range(2):
        for hl in range(64):
            hg = hbase + rt * 64 + hl
            for i in range(5):
                r = _reflect_idx(hg - 2 + i, H)
                j = r + 2 - hbase          # canonical shard row
                M[j - rt * 64, rt * 64 + hl] += 1.0
    return M


def _ksegs_in_octet(oct_idx):
    """Pair channels live at ch 16..151. For a channel octet [oct*8, oct*8+8),
    return list of (j0, k, l0, nl): local offset j0, channel k, first l, count."""
    lo, hi = oct_idx * 8, oct_idx * 8 + 8
    segs = []
    p = 0
    for k in range(K):
        n = K - k
        # pairs (k, k..15) occupy ch [16+p, 16+p+n)
        s, e = 16 + p, 16 + p + n
        a, b = max(lo, s), min(hi, e)
        if a < b:
            segs.append((a - lo, k, k + (a - s), b - a))
        p += n
    return segs


def _build_kernel(t_bufs=3, m_bufs=2, r_bufs=2, i1_bufs=2):
    nc = bacc.Bacc("TRN2", target_bir_lowering=False, debug=False)
    S_d = nc.dram_tensor("S", [SR, K, W], BF16, kind="ExternalInput").ap()
    BR_d = nc.dram_tensor("BR", [68, 128], BF16, kind="ExternalInput").ap()
    BW_d = nc.dram_tensor("BW", [128, 512], BF16, kind="ExternalInput").ap()
    R_d = nc.dram_tensor("R", [HH, W, K * K], BF16, kind="ExternalOutput").ap()

    with tile.TileContext(nc) as tc, ExitStack() as ctx:
        const_p = ctx.enter_context(tc.tile_pool(name="const", bufs=1))
        sp_p = ctx.enter_context(tc.tile_pool(name="sp", bufs=1))
        t_p = ctx.enter_context(tc.tile_pool(name="tprod", bufs=t_bufs))
        i1_p = ctx.enter_context(tc.tile_pool(name="i1", bufs=i1_bufs))
        mu_p = ctx.enter_context(tc.tile_pool(name="mu", bufs=2))
        m_p = ctx.enter_context(tc.tile_pool(name="mm", bufs=m_bufs))
        r_p = ctx.enter_context(tc.tile_pool(name="rout", bufs=r_bufs))
        ps1_p = ctx.enter_context(tc.tile_pool(name="ps1", bufs=2, space="PSUM"))
        ps2_p = ctx.enter_context(tc.tile_pool(name="ps2", bufs=4, space="PSUM"))

        br = const_p.tile([68, 128], BF16)
        bw = const_p.tile([128, 512], BF16)
        nc.sync.dma_start(br[:], BR_d)
        nc.sync.dma_start(bw[:], BW_d)

        sp0 = sp_p.tile([68, K, W], BF16)
        sp1 = sp_p.tile([68, K, W], BF16)
        nc.sync.dma_start(sp0[:], S_d[0:68])
        nc.sync.dma_start(sp1[:], S_d[64:132])
        sps = [sp0, sp1]

        for rt in range(2):
            sp = sps[rt]
            brt = br[:, rt * 64:(rt + 1) * 64]
            i1c0 = i1_p.tile([128, NCH * 64], BF16, name="i1c0")
            i1c1 = i1_p.tile([128, NCH * 64], BF16, name="i1c1")
            for oc in range(NOCT):
                T = t_p.tile([68, 8, W], BF16, name="T")
                lo = oc * 8
                if lo < K:  # mean channels: plain copy
                    n = min(8, K - lo)
                    nc.vector.tensor_copy(T[:, 0:n, :], sp[:, lo:lo + n, :])
                for (j0, k, l0, nl) in _ksegs_in_octet(oc):
                    in0 = sp[:, k, :].unsqueeze(1).broadcast_to([68, nl, W])
                    nc.vector.tensor_mul(
                        T[:, j0:j0 + nl, :], in0, sp[:, l0:l0 + nl, :])
                psa = ps1_p.tile([128, 512], F32, name="psa")
                psb = ps1_p.tile([128, 512], F32, name="psb")
                Tf = T[:].rearrange("p a b -> p (a b)")
                for j in range(8):
                    nc.tensor.matmul(psa[:, j * 64:(j + 1) * 64],
                                     Tf[:, j * 256:j * 256 + 128], brt,
                                     start=True, stop=True)
                    nc.tensor.matmul(psb[:, j * 64:(j + 1) * 64],
                                     Tf[:, j * 256 + 128:j * 256 + 256], brt,
                                     start=True, stop=True)
                nc.scalar.copy(i1c0[:, oc * 512:(oc + 1) * 512], psa[:])
                nc.scalar.copy(i1c1[:, oc * 512:(oc + 1) * 512], psb[:])

            i0v = i1c0[:].rearrange("p (c h) -> p c h", c=NCH)
            i1v = i1c1[:].rearrange("p (c h) -> p c h", c=NCH)
            for oh in range(2):
                bwa = bw[:, (oh * 2) * 128:(oh * 2) * 128 + 128]
                bwb = bw[:, (oh * 2 + 1) * 128:(oh * 2 + 1) * 128 + 128]
                # --- mean channels -> mu (bf16, scaled by 1/5) ---
                mub = mu_p.tile([128, K, 64], BF16, name="mub")
                for mo in range(2):
                    ps2 = ps2_p.tile([128, 512], F32, name="ps2")
                    nc.tensor.matmul(ps2[:], bwa,
                                     i0v[:, mo * 8:(mo + 1) * 8, :],
                                     start=True, stop=False)
                    nc.tensor.matmul(ps2[:], bwb,
                                     i1v[:, mo * 8:(mo + 1) * 8, :],
                                     start=False, stop=True)
                    nc.scalar.mul(mub[:, mo * 8:(mo + 1) * 8, :],
                                  ps2[:].rearrange("p (c h) -> p c h", c=8), 0.2)
                # --- M = mu_k * mu_l ---
                M = m_p.tile([128, NPAIR, 64], BF16, name="M")
                p0 = 0
                for k in range(K):
                    nl = K - k
                    in0 = mub[:, k, :].unsqueeze(1).broadcast_to([128, nl, 64])
                    nc.vector.tensor_mul(M[:, p0:p0 + nl, :], in0,
                                         mub[:, k:K, :])
                    p0 += nl
                # --- pair channels: MM2, subtract, mirror ---
                rsb = r_p.tile([128, 64, K * K], BF16, name="rsb")
                for oc in range(2, NOCT):
                    ps2 = ps2_p.tile([128, 512], F32, name="ps2")
                    nc.tensor.matmul(ps2[:], bwa,
                                     i0v[:, oc * 8:(oc + 1) * 8, :],
                                     start=True, stop=False)
                    nc.tensor.matmul(ps2[:], bwb,
                                     i1v[:, oc * 8:(oc + 1) * 8, :],
                                     start=False, stop=True)
                    p2v = ps2[:].rearrange("p (c h) -> p c h", c=8)
                    for (j0, k, l0, nl) in _ksegs_in_octet(oc):
                        pr = (l0 - k) + (k * (2 * K + 1 - k)) // 2
                        # out positions kl = k*16 + l, l in [l0, l0+nl)
                        dst = rsb[:].rearrange("p h q -> p q h")[
                            :, k * K + l0:k * K + l0 + nl, :]
                        nc.vector.tensor_sub(dst, p2v[:, j0:j0 + nl, :],
                                             M[:, pr:pr + nl, :])
                # mirror lower triangle on gpsimd
                rq = rsb[:].rearrange("p h q -> p q h")
                rqv = rsb[:].rearrange("p h (a b) -> p a b h", a=K)
                for k in range(K - 1):
                    src = rq[:, k * K + k + 1:k * K + K, :]
                    dst = rqv[:, k + 1:K, k, :]
                    nc.gpsimd.tensor_copy(dst, src)
                # DMA out: R[rt*64 : +64, oh*128 : +128, :]
                dview = R_d[rt * 64:rt * 64 + 64,
                            oh * 128:(oh + 1) * 128, :].transpose([1, 0, 2])
                nc.sync.dma_start(dview, rsb[:])

    nc.compile()
    return nc


_NC_CACHE = {}


def _get_nc():
    if "nc" not in _NC_CACHE:
        _NC_CACHE["nc"] = _build_kernel()
    return _NC_CACHE["nc"]


def _prep_in_maps(S):
    S = np.asarray(S, dtype=np.float32)
    np_bf16 = mybir.dt.np(BF16)
    bw = _build_bw().astype(np_bf16)
    brs = [(_build_br(h)).astype(np_bf16) for h in range(2)]
    Ss = S * np.float32(0.2)
    in_maps = []
    for b in range(B):
        for half in range(2):
            hbase = half * HH
            rows = np.clip(np.arange(hbase - 2, hbase + 130), 0, H - 1)
            shard = Ss[b][:, rows, :].transpose(1, 0, 2)   # [132, K, 256]
            shard = np.ascontiguousarray(shard).astype(np_bf16)
            in_maps.append({"S": shard, "BR": brs[half], "BW": bw})
    return in_maps


def _assemble(results):
    out = np.empty((B, H, W, K, K), dtype=np.float32)
    for i in range(8):
        b, half = divmod(i, 2)
        r = np.asarray(results[i]["R"]).astype(np.float32)
        out[b, half * HH:(half + 1) * HH] = r.reshape(HH, W, K, K)
    return out


def kernel(S):
    """S: [4, 16, 256, 256] float32 -> R: [4, 256, 256, 16, 16] float32."""
    nc = _get_nc()
    in_maps = _prep_in_maps(S)
    res = bass_utils.run_bass_kernel_spmd(nc, in_maps, list(range(8)))
    return _assemble(res.results)
